# revision 36
# baseline (speedup 1.0000x reference)
"""Trainium2 Bass kernel for nn_Attn: softmax(enc @ (W^T h)) over seq_len.

Math: energy = enc @ W^T + b; attn = energy @ h; out = softmax(attn).
Algebraically attn[s] = enc[s,:] . v + (b.h) with v = W^T h, and the (b.h)
term is constant across s so softmax cancels it. The device work is the
memory-bound part: streaming the 128 MiB encoder_outputs once, sharded
along seq_len across 8 NeuronCores. Per 128-row block: VectorE multiplies
by v (tensor_tensor), ScalarE reduces rows (activation Copy + accum_out),
so the two passes over the data run on different engines concurrently.
"""
import numpy as np

S = 32768
H = 1024
N_CORES = 8
S_SHARD = S // N_CORES          # 4096 rows per core
P = 128                         # partitions
N_BLK = S_SHARD // P            # 32 row-blocks per core
# blocks per dma_start: small at the edges (fast pipeline rampup/drain),
# 2 MiB in the middle (DMA efficiency); covers blocks 0..N_BLK-2. The final
# block is streamed as two H-halves so its DMA lands earlier and its
# mult+reduce chain (the only compute on the critical path after the last
# byte arrives) is half as long; the host adds the two partial sums.
DMA_SCHED = [1, 1] + [2] * 14 + [1]
E_CHUNKS = 4                    # output DMA'd in column chunks as it completes

_cache = {}


def _build():
    from concourse import bacc, mybir, tile

    nc = bacc.Bacc("TRN2", target_bir_lowering=False, debug=False,
                   num_devices=N_CORES)
    enc = nc.dram_tensor("enc", [S_SHARD, H], mybir.dt.float32,
                         kind="ExternalInput")
    v_in = nc.dram_tensor("v_in", [1, H], mybir.dt.float32,
                          kind="ExternalInput")
    e_out = nc.dram_tensor("e_out", [P, N_BLK + 1], mybir.dt.float32,
                           kind="ExternalOutput")

    ECW = N_BLK // E_CHUNKS     # columns per output chunk

    with tile.TileContext(nc) as tc:
        with tc.tile_pool(name="const", bufs=1) as cpool, \
             tc.tile_pool(name="psum", bufs=1, space="PSUM") as qpool, \
             tc.tile_pool(name="stream", bufs=8) as spool, \
             tc.tile_pool(name="prod", bufs=4) as ppool, \
             tc.tile_pool(name="cpout", bufs=4) as opool:
            # vt = ones[P,1] @ v[1,H] on PE: avoids streaming 0.5 MB of
            # host-replicated v through the shared HBM stack
            v0 = cpool.tile([1, H], mybir.dt.float32)
            nc.gpsimd.dma_start(out=v0[:], in_=v_in.ap())
            ones = cpool.tile([1, P], mybir.dt.float32)
            nc.vector.memset(ones[:], 1.0)
            pv = qpool.tile([P, H], mybir.dt.float32)
            nc.tensor.matmul(out=pv[:, 0:512], lhsT=ones[:],
                             rhs=v0[:, 0:512], start=True, stop=True)
            nc.tensor.matmul(out=pv[:, 512:H], lhsT=ones[:],
                             rhs=v0[:, 512:H], start=True, stop=True)
            vt = cpool.tile([P, H], mybir.dt.float32)
            nc.scalar.copy(out=vt[:, 0:512], in_=pv[:, 0:512])
            nc.scalar.copy(out=vt[:, 512:H], in_=pv[:, 512:H])
            Es = [cpool.tile([P, ECW], mybir.dt.float32, tag=f"E{k}",
                             name=f"E{k}") for k in range(E_CHUNKS)]
            b0 = 0
            for nb in DMA_SCHED:
                t = spool.tile([P, nb, H], mybir.dt.float32, tag=f"t{nb}")
                rows = enc.ap()[b0 * P:(b0 + nb) * P, :]
                nc.sync.dma_start(out=t[:],
                                  in_=rows.rearrange("(i p) h -> p i h", p=P))
                for i in range(nb):
                    b = b0 + i
                    prod = ppool.tile([P, H], mybir.dt.float32, tag="prod")
                    nc.vector.tensor_tensor(out=prod[:], in0=t[:, i, :],
                                            in1=vt[:],
                                            op=mybir.AluOpType.mult)
                    Et, col = Es[b // ECW], b % ECW
                    cp = opool.tile([P, H], mybir.dt.float32, tag="cp")
                    nc.scalar.activation(
                        out=cp[:], in_=prod[:],
                        func=mybir.ActivationFunctionType.Copy,
                        accum_out=Et[:, col:col + 1])
                b0 += nb
            # final block, split into H-halves; partial sums go to the last
            # two output columns (N_BLK-1 and N_BLK), host adds them
            HH = H // 2
            last = (N_BLK - 1) * P
            Ef = cpool.tile([P, 2], mybir.dt.float32)
            th0 = spool.tile([P, HH], mybir.dt.float32, tag="th")
            th1 = spool.tile([P, HH], mybir.dt.float32, tag="th")
            nc.sync.dma_start(out=th0[:], in_=enc.ap()[last:, 0:HH])
            nc.sync.dma_start(out=th1[:], in_=enc.ap()[last:, HH:H])
            ph0 = ppool.tile([P, HH], mybir.dt.float32, tag="ph")
            nc.vector.tensor_tensor(out=ph0[:], in0=th0[:], in1=vt[:, 0:HH],
                                    op=mybir.AluOpType.mult)
            ph1 = ppool.tile([P, HH], mybir.dt.float32, tag="ph")
            nc.vector.tensor_tensor(out=ph1[:], in0=th1[:], in1=vt[:, HH:H],
                                    op=mybir.AluOpType.mult)
            cpf = opool.tile([P, HH], mybir.dt.float32, tag="cpf")
            nc.scalar.activation(out=cpf[:], in_=ph0[:],
                                 func=mybir.ActivationFunctionType.Copy,
                                 accum_out=Ef[:, 0:1])
            nc.vector.tensor_reduce(out=Ef[:, 1:2], in_=ph1[:],
                                    axis=mybir.AxisListType.X,
                                    op=mybir.AluOpType.add)
            for k in range(E_CHUNKS - 1):
                nc.sync.dma_start(out=e_out.ap()[:, k * ECW:(k + 1) * ECW],
                                  in_=Es[k][:])
            # last chunk stops before col N_BLK-1; the final block's two
            # partial sums own cols N_BLK-1 and N_BLK
            nc.sync.dma_start(
                out=e_out.ap()[:, (E_CHUNKS - 1) * ECW:N_BLK - 1],
                in_=Es[E_CHUNKS - 1][:, 0:ECW - 1])
            nc.sync.dma_start(out=e_out.ap()[:, N_BLK - 1:N_BLK + 1],
                              in_=Ef[:])
    nc.compile()
    return nc


def _get_nc():
    if "nc" not in _cache:
        _cache["nc"] = _build()
    return _cache["nc"]


def kernel(hidden, encoder_outputs, W, b):
    from concourse import bass_utils

    nc = _get_nc()
    h = np.asarray(hidden, dtype=np.float32)[0]
    enc = np.ascontiguousarray(np.asarray(encoder_outputs,
                                          dtype=np.float32)[:, 0, :])
    v = (np.asarray(W, dtype=np.float32).T @ h).astype(np.float32)

    in_maps = [{"enc": enc[c * S_SHARD:(c + 1) * S_SHARD],
                "v_in": v[None, :]} for c in range(N_CORES)]
    res = bass_utils.run_bass_kernel_spmd(
        nc, in_maps, core_ids=list(range(N_CORES)),
        trace=_cache.get("trace", False))
    _cache["last_result"] = res

    # e_out is [partition, block] plus an extra column holding the second
    # partial sum of the final block; global row s = core*4096 + block*128 + p.
    shards = []
    for c in range(N_CORES):
        eo = res.results[c]["e_out"]
        eb = eo[:, :N_BLK].copy()
        eb[:, N_BLK - 1] += eo[:, N_BLK]
        shards.append(eb.T.reshape(S_SHARD))
    e = np.concatenate(shards)
    e = e - e.max()
    p = np.exp(e)
    out = (p / p.sum()).astype(np.float32)
    return out[None, None, :]
